# revision 28
# baseline (speedup 1.0000x reference)
"""Trainium2 Bass kernel for nn_CausalityMapBlock (raw bass, manual sync).

Math: with p = 1.0 and EPS = 1e-8 the whole block collapses to rank-1.
For xf = x/(max+EPS), S1 = sum(xf), S2 = sum(xf^2) per channel:
  lehmer_num[m,n] ~= S2[m]S2[n]/(S1[m]S1[n]),  lehmer_den[n] ~= S2[n]/S1[n]
  out[m,n] = lehmer_num/lehmer_den ~= S2[m]/S1[m]   (constant along n)
All EPS correction terms are O(1e-7) relative (verified 1.3e-6 max rel
err vs the reference).  With raw sums t = sum(x), q = sum(x^2):
  out[m,n] = q[m] / (t[m] * gmax)

Layout strategy: everything per-channel stays in [128,1] columns (128
lanes parallel); only the global max and the matmul lhsT cross the
partition axis, via one PE transpose of packed stat columns. Row ops
on [1,p] tiles are single-lane on DVE (~7ns/elem), so the only row
work is one reduce_max and a [1,1] reciprocal; 1/t is applied as a
per-partition AP scalar during the final PSUM->SBUF copy, and 1/gmax
as the ACT scale of the lhsT row copy. Stat columns sit 32 apart so
the transposed rows land on legal 32-aligned partitions.

Raw bass (no Tile framework): manual semaphores avoid Tile's startup
barrier and teardown sem-reset storm. Each instruction carries at most
one embedded wait (walrus limit); extra deps use standalone waits.

Sharding: data-parallel over batch B=2; cores 0-3 compute batch 0,
cores 4-7 batch 1 (redundantly within a group; wall-clock identical).
"""

import sys

import numpy as np

for _p in ("/opt/trn_rl_repo",):
    if _p not in sys.path:
        sys.path.insert(0, _p)

EPS = 1e-8
B, C, H, W = 2, 128, 7, 7
F = H * W  # 49
N_CORES = 8

_CACHE = {}


def _build_nc():
    import concourse.bacc as bacc
    import concourse.mybir as mybir

    fp32 = mybir.dt.float32
    bf16 = mybir.dt.bfloat16
    MUL = mybir.AluOpType.mult
    AX = mybir.AxisListType.X
    COPY = mybir.ActivationFunctionType.Copy

    nc = bacc.Bacc("TRN2", target_bir_lowering=False, debug=False)
    xb = nc.dram_tensor("xb", [C, F], fp32, kind="ExternalInput")
    out = nc.dram_tensor("out", [C, C], fp32, kind="ExternalOutput")

    from contextlib import ExitStack

    with ExitStack() as ctx:
        sb = lambda name, shape, dt=fp32: ctx.enter_context(
            nc.sbuf_tensor(name, shape, dt)
        )
        ps = lambda name, shape: ctx.enter_context(
            nc.psum_tensor(name, shape, fp32)
        )
        ident = sb("ident", [128, 128])
        X = sb("X", [C, F])
        X2 = sb("X2", [C, F])      # DVE stt main output (q accum side)
        XJ = sb("XJ", [C, F])      # ACT copy main output (t accum side)
        # stat columns 32 apart: col 0 = mt (per-channel max), col 32 = q
        Wst = sb("Wst", [C, 33])
        tcol = sb("tcol", [C, 1])  # t = sum(x) per channel (from ACT)
        rtc = sb("rtc", [C, 1])    # 1/t column
        onesb = sb("onesb", [1, 128], bf16)
        gmax = sb("gmax", [1, 1])
        sv = sb("sv", [1, 1])
        urow = sb("urow", [1, 128], bf16)
        osb = sb("osb", [128, 128])
        jnk = sb("jnk", [1, 1])
        TP = ps("TP", [33, 128])
        OPS = ps("OPS", [128, 128])
        dma_sem = ctx.enter_context(nc.semaphore("dma_sem"))
        dve_sem = ctx.enter_context(nc.semaphore("dve_sem"))
        act_sem = ctx.enter_context(nc.semaphore("act_sem"))
        pe_sem = ctx.enter_context(nc.semaphore("pe_sem"))
        pool_sem = ctx.enter_context(nc.semaphore("pool_sem"))
        block = ctx.enter_context(nc.Block(no_gpsimd_drain=True))

        @block.sync
        def _(sync):
            # input/output DMAs split across the two HWDGE queues (SP +
            # ACT) — per-partition packet overhead dominates
            sync.dma_start(X[0:64, :], xb.ap()[0:64, :]).then_inc(
                dma_sem, 16
            )
            sync.wait_ge(dve_sem, 5)
            sync.wait_ge(act_sem, 3)
            # no completion wait on the output DMAs: NRT drains the HWDGE
            # rings before signaling NEFF completion (incs required by
            # codegen; next run's preamble clears them)
            sync.dma_start(out.ap()[0:64, :], osb[0:64, :]).then_inc(
                dma_sem, 16
            )

        @block.scalar
        def _(scalar):
            scalar.dma_start(X[64:128, :], xb.ap()[64:128, :]).then_inc(
                dma_sem, 16
            )
            # dummy activation: absorbs the one-time ACT table load while
            # the kernel is still waiting on the input DMA
            nc.scalar.copy(jnk[:], onesb[0:1, 0:1])._wait_ge(dve_sem, 1)
            # t = sum(x) per channel via Copy-with-accum, in parallel
            # with DVE's max/sumsq reduces
            nc.scalar.activation(
                XJ[:], X[:], COPY, accum_out=tcol[:]
            )._wait_ge(dma_sem, 32).then_inc(act_sem, 1)
            # lhsT of the broadcast matmul: u_row = q_row * (1/gmax),
            # scaled during the PSUM->SBUF row copy (bf16 out); runs on
            # ACT so DVE's chain ends at sv
            nc.scalar.activation(
                urow[:], TP[32:33, :], COPY, scale=sv[:]
            )._wait_ge(dve_sem, 4).then_inc(act_sem, 1)
            # right column half of the final scale-copy (free-size sets
            # the op cost, so column-splitting across ACT+DVE halves it)
            nc.scalar.activation(
                osb[:, 64:128], OPS[:, 64:128], COPY, scale=rtc[:]
            )._wait_ge(pe_sem, 2).then_inc(act_sem, 1)
            scalar.wait_ge(dve_sem, 5)
            scalar.dma_start(out.ap()[64:128, :], osb[64:128, :])._wait_ge(
                act_sem, 3
            ).then_inc(dma_sem, 16)

        @block.gpsimd
        def _(gpsimd):
            # identity for the PE transpose; entirely off the critical
            # path (runs during the input-DMA wait)
            nc.gpsimd.memset(ident[:], 0.0)
            nc.gpsimd.drain()
            nc.gpsimd.affine_select(
                out=ident[:], in_=ident[:],
                compare_op=mybir.AluOpType.not_equal,
                fill=1.0, base=0,
                pattern=[[-1, 128]], channel_multiplier=1,
            ).then_inc(pool_sem, 1)

        @block.vector
        def _(vector):
            nc.vector.memset(onesb[:], 1.0).then_inc(dve_sem, 1)
            # zero unused stat columns so the transpose never reads
            # uninitialized SBUF (rows 1-31 of TP are never used)
            nc.vector.memset(Wst[:], 0.0)
            # per-channel stats (column layout, 128-lane parallel)
            nc.vector.reduce_max(Wst[:, 0:1], X[:], axis=AX)._wait_ge(
                dma_sem, 32
            )
            nc.vector.scalar_tensor_tensor(
                X2[:], X[:], 1.0, X[:], op0=MUL, op1=MUL,
                accum_out=Wst[:, 32:33],
            ).then_inc(dve_sem, 1)  # dve=2 -> PE transpose go
            nc.vector.reduce_max(gmax[:], TP[0:1, :], axis=AX)._wait_ge(
                pe_sem, 1
            )
            # 1/t column (parallel lanes): consumed by the final
            # scale-copy; doubles as the gmax->sv RAW spacer
            nc.vector.reciprocal(rtc[:], tcol[:])._wait_ge(
                act_sem, 1
            ).then_inc(dve_sem, 1)  # dve=3
            nc.vector.reciprocal(sv[:], gmax[:]).then_inc(
                dve_sem, 1
            )  # dve=4 -> ACT urow copy go
            # osb[m,n] = OPS[m,n] * (1/t[m]): per-partition AP scalar
            # broadcast along free; left column half (ACT does the right)
            nc.vector.tensor_scalar_mul(
                osb[:, 0:64], OPS[:, 0:64], rtc[:]
            )._wait_ge(pe_sem, 2).then_inc(dve_sem, 1)  # dve=5 -> out DMAs

        @block.tensor
        def _(tensor):
            tensor.wait_ge(pool_sem, 1)
            nc.tensor.transpose(TP[:], Wst[:, 0:33], ident[:])._wait_ge(
                dve_sem, 2
            ).then_inc(pe_sem, 1)
            # K=1 bf16 outer product broadcasts u down the [C,C] grid:
            # OPS[m,n] = urow[m]
            nc.tensor.matmul(
                OPS[:], urow[:], onesb[:], start=True, stop=True,
            )._wait_ge(act_sem, 2).then_inc(pe_sem, 1)

    nc.compile()
    return nc


def _get_nc():
    if "nc" not in _CACHE:
        _CACHE["nc"] = _build_nc()
    return _CACHE["nc"]


def kernel(x) -> np.ndarray:
    from concourse.bass_utils import run_bass_kernel_spmd

    x = np.ascontiguousarray(np.asarray(x), dtype=np.float32)
    assert x.shape == (B, C, H, W)
    xf = x.reshape(B, C, F)

    nc = _get_nc()
    in_maps = [{"xb": np.ascontiguousarray(xf[i // 4])} for i in range(N_CORES)]
    try:
        res = run_bass_kernel_spmd(nc, in_maps, list(range(N_CORES))).results
    except Exception:
        # transient NRT/device hiccups recover on a clean retry
        res = run_bass_kernel_spmd(nc, in_maps, list(range(N_CORES))).results
    return np.stack([res[0]["out"], res[4]["out"]]).astype(np.float32)
